# revision 2
# baseline (speedup 1.0000x reference)
"""Trainium2 Bass kernel for DisturbanceRegressionLoss2Heads.

Reference computation (per batch element b, per pixel (h, w)):
  y0 = out[b, 0]  (Y=30 time steps)   y1 = out[b, 1]
  diff = [-7, 0, y0[2]-y0[1], ..., y0[28]-y0[27], 0]
  d = argmin(diff)  (first min)
  piecewise OLS fit of y0 over t<d (x=t) and t>=d (x=t-d), slopes clipped to
  [0,2] in the fitted line, intercepts clipped to [0,100]
  loss = mean over everything of (fitted - y1)^2

Strategy: pure data parallel over the batch (8 cores, one batch element each).
Per core, pixels are tiled [128 partitions x F pixels/partition] and the
30-step time axis lives in the free dimension; chunks are double-buffered so
DMA and the three compute engines overlap.  The argmin is one running-min
tensor_tensor_scan over pixel-major diffs with multiply-by-zero boundary
resets; mask(t<d) falls out as (running_min != final_min); segment sums come
from masked products + innermost-axis reduces; a per-pixel OLS epilogue forms
clipped slopes/intercepts; the piecewise-fitted curve is assembled with
broadcast APs + copy_predicated and the squared residual is accumulated
per-partition by the scalar engine's activation(Square, accum_out).
Each core writes 128 x NCHUNK partial sums; the host sums them in float64.

Built with bacc.Bacc: its generate_event_semaphores pass splits multi-wait
sync (TRN2 allows at most one wait per instruction) — raw bass.Bass kernels
of this shape fail walrus codegen or corrupt sync and hang the device.
"""

import numpy as np

import concourse.bacc as bacc
import concourse.tile as tile
from concourse import mybir
from concourse.bass_utils import run_bass_kernel_spmd

F32 = mybir.dt.float32
BF16 = mybir.dt.bfloat16
AX = mybir.AxisListType
OP = mybir.AluOpType
AF = mybir.ActivationFunctionType

B = 8
Y = 30
H = 256
W = 256
NPIX = H * W          # 65536 pixels per core
P = 128               # SBUF partitions
FP = NPIX // P        # 512 pixels per partition, whole core
F = 128               # pixels per partition per chunk
CHUNK = P * F
NCHUNK = FP // F      # 4
DIST = 7.0
MAXI = 100.0

# engine assignment config (tuned against the TimelineSim cost model)
CFG = {"diff_g": False, "pbt_g": True, "y0t_g": True, "fb_g": True,
       "neq_g": False, "fa_g": False, "r_g": True,
       "io_bufs": 2, "work_bufs": 2, "sm_bufs": 2, "fa_tag": "dpt"}


def _emit_chunk(nc, pools, c, y0d, y1d, z, tb, partial):
    """One chunk: argmin, masked sums, OLS epilogue, fitted curve, residual."""
    io, work, sm = pools
    cs = c * CHUNK

    y0a = io.tile([P, Y, F], F32, tag="y0a")
    y1a = io.tile([P, Y, F], F32, tag="y1a")
    nc.sync.dma_start(out=y0a[:],
                      in_=y0d[:, cs:cs + CHUNK].rearrange("y (p f) -> p y f",
                                                          p=P))
    nc.sync.dma_start(out=y1a[:],
                      in_=y1d[:, cs:cs + CHUNK].rearrange("y (p f) -> p y f",
                                                          p=P))
    y0pt = y0a[:].rearrange("p t f -> p f t")   # [P, F, Y] strided view
    y1pt = y1a[:].rearrange("p t f -> p f t")

    def b3(small):  # broadcast a [P, F] per-pixel tile along t
        return small[:][:, :, None].broadcast_to([P, F, Y])

    # ---- modified diff array, pixel-major (t contiguous per pixel)
    dpt = work.tile([P, F, Y], F32, tag="dpt")
    deng = nc.gpsimd if CFG["diff_g"] else nc.vector
    deng.tensor_tensor(
        out=dpt[:, :, 2:29], in0=y0pt[:, :, 2:29], in1=y0pt[:, :, 1:28],
        op=OP.subtract)
    deng.memset(dpt[:, :, 0:1], -DIST)
    deng.memset(dpt[:, :, 1:2], 0.0)
    deng.memset(dpt[:, :, 29:30], 0.0)

    # ---- running min along t (reset at pixel boundaries via z=0 slots)
    M = work.tile([P, F, Y], F32, tag="M")
    nc.vector.tensor_tensor_scan(
        out=M[:].rearrange("p f t -> p (f t)"),
        data0=z[:].rearrange("p f t -> p (f t)"),
        data1=dpt[:].rearrange("p f t -> p (f t)"),
        initial=0.0, op0=OP.mult, op1=OP.min)

    # ---- final min per pixel, then maskB = [t < d] = (runmin != finalmin)
    mst = sm.tile([P, F], F32, tag="mst")
    nc.vector.tensor_copy(mst[:], M[:, :, Y - 1])
    maskB = work.tile([P, F, Y], F32, tag="maskB")
    neng = nc.gpsimd if CFG["neq_g"] else nc.vector
    neng.tensor_tensor(out=maskB[:], in0=M[:], in1=b3(mst),
                       op=OP.not_equal)

    # ---- d (= n_before) and masked/unmasked first-order sums over t
    d = sm.tile([P, F], F32, tag="d")
    nc.vector.tensor_reduce(out=d[:], in_=maskB[:], axis=AX.X, op=OP.add)

    pb = work.tile([P, F, Y], F32, tag="dpt")     # reuse dpt slot group
    nc.vector.tensor_tensor(out=pb[:], in0=maskB[:], in1=y0pt, op=OP.mult)
    syb = sm.tile([P, F], F32, tag="syb")
    nc.vector.tensor_reduce(out=syb[:], in_=pb[:], axis=AX.X, op=OP.add)

    # t-weighted masked sum: multiply pb by t in place, then reduce
    peng = nc.gpsimd if CFG["pbt_g"] else nc.vector
    peng.tensor_tensor(out=pb[:], in0=pb[:], in1=tb, op=OP.mult)
    styb = sm.tile([P, F], F32, tag="styb")
    nc.vector.tensor_reduce(out=styb[:], in_=pb[:], axis=AX.X, op=OP.add)

    # totals: reduce y0, then scale y0 by t in place (its last use), reduce
    ty = sm.tile([P, F], F32, tag="ty")
    nc.vector.tensor_reduce(out=ty[:], in_=y0pt, axis=AX.X, op=OP.add)
    yeng = nc.gpsimd if CFG["y0t_g"] else nc.vector
    yeng.tensor_tensor(out=y0pt, in0=y0pt, in1=tb, op=OP.mult)
    tty = sm.tile([P, F], F32, tag="tty")
    nc.vector.tensor_reduce(out=tty[:], in_=y0pt, axis=AX.X, op=OP.add)

    # ---- per-pixel regression epilogue ([P, F] smalls)
    def tt(name, a, bb, op):
        t = sm.tile([P, F], F32, tag=name)
        nc.vector.tensor_tensor(out=t[:], in0=a[:], in1=bb[:], op=op)
        return t

    na = sm.tile([P, F], F32, tag="na")           # 30 - d
    nc.scalar.activation(out=na[:], in_=d[:], func=AF.Copy, bias=float(Y),
                         scale=-1.0)
    sya = tt("sya", ty, syb, OP.subtract)         # sum y, t>=d
    t0 = tt("t0", tty, styb, OP.subtract)         # sum t*y, t>=d
    t1 = tt("t1", d, sya, OP.mult)
    nc.vector.tensor_tensor(out=t0[:], in0=t0[:], in1=t1[:], op=OP.subtract)
    sxya = t0                                     # sum (t-d)*y, t>=d

    nbs = sm.tile([P, F], F32, tag="nbs")
    nc.vector.tensor_scalar(out=nbs[:], in0=d[:], scalar1=1.0, scalar2=None,
                            op0=OP.max)
    nc.vector.reciprocal(out=nbs[:], in_=nbs[:])
    ra = sm.tile([P, F], F32, tag="ra")
    nc.vector.reciprocal(out=ra[:], in_=na[:])

    myb = tt("myb", syb, nbs, OP.mult)            # mean y before
    mya = tt("mya", sya, ra, OP.mult)             # mean y after
    mxb = sm.tile([P, F], F32, tag="mxb")         # (d-1)/2
    nc.scalar.activation(out=mxb[:], in_=d[:], func=AF.Copy, bias=-0.5,
                         scale=0.5)
    mxa = sm.tile([P, F], F32, tag="mxa")         # (na-1)/2
    nc.scalar.activation(out=mxa[:], in_=na[:], func=AF.Copy, bias=-0.5,
                         scale=0.5)

    covb = tt("covb", mxb, syb, OP.mult)          # mxb*syb, then styb - that
    nc.vector.tensor_tensor(out=covb[:], in0=styb[:], in1=covb[:],
                            op=OP.subtract)
    cova = tt("cova", mxa, sya, OP.mult)
    nc.vector.tensor_tensor(out=cova[:], in0=sxya[:], in1=cova[:],
                            op=OP.subtract)

    # var*12 = n*(n^2-1); slope = cov / max(var, 1) gated on var > 0
    vb12 = tt("vb12", d, d, OP.mult)
    nc.vector.scalar_tensor_tensor(out=vb12[:], in0=vb12[:], scalar=1.0,
                                   in1=d[:], op0=OP.subtract, op1=OP.mult)
    mvb = sm.tile([P, F], F32, tag="mvb")
    nc.vector.tensor_scalar(out=mvb[:], in0=vb12[:], scalar1=1.0 / 12.0,
                            scalar2=1.0, op0=OP.mult, op1=OP.max)
    nc.vector.reciprocal(out=mvb[:], in_=mvb[:])
    slb = tt("slb", covb, mvb, OP.mult)
    nc.vector.tensor_scalar(out=vb12[:], in0=vb12[:], scalar1=0.0, scalar2=None,
                            op0=OP.is_gt)        # gate, reuses vb12
    nc.vector.tensor_tensor(out=slb[:], in0=slb[:], in1=vb12[:], op=OP.mult)

    va12 = tt("va12", na, na, OP.mult)
    nc.vector.scalar_tensor_tensor(out=va12[:], in0=va12[:], scalar=1.0,
                                   in1=na[:], op0=OP.subtract, op1=OP.mult)
    nc.vector.tensor_scalar(out=va12[:], in0=va12[:], scalar1=1.0 / 12.0,
                            scalar2=1.0, op0=OP.mult, op1=OP.max)
    nc.vector.reciprocal(out=va12[:], in_=va12[:])
    sla = tt("sla", cova, va12, OP.mult)          # slope after (na>=2 always)

    # intercepts use the *unclipped* slope; fitted lines use clipped slopes
    ibv = tt("ibv", slb, mxb, OP.mult)
    nc.vector.tensor_tensor(out=ibv[:], in0=myb[:], in1=ibv[:], op=OP.subtract)
    nc.vector.tensor_scalar(out=ibv[:], in0=ibv[:], scalar1=0.0, scalar2=MAXI,
                            op0=OP.max, op1=OP.min)
    iav = tt("iav", sla, mxa, OP.mult)
    nc.vector.tensor_tensor(out=iav[:], in0=mya[:], in1=iav[:], op=OP.subtract)
    nc.vector.tensor_scalar(out=iav[:], in0=iav[:], scalar1=0.0, scalar2=MAXI,
                            op0=OP.max, op1=OP.min)
    sbc = sm.tile([P, F], F32, tag="sbc")
    nc.vector.tensor_scalar(out=sbc[:], in0=slb[:], scalar1=0.0, scalar2=2.0,
                            op0=OP.max, op1=OP.min)
    sac = sm.tile([P, F], F32, tag="sac")
    nc.vector.tensor_scalar(out=sac[:], in0=sla[:], scalar1=0.0, scalar2=2.0,
                            op0=OP.max, op1=OP.min)
    ia2f = tt("ia2f", sac, d, OP.mult)            # ia - sac*d
    nc.vector.tensor_tensor(out=ia2f[:], in0=iav[:], in1=ia2f[:],
                            op=OP.subtract)

    # ---- fitted curve: fa = sac*t + ia2f, overwritten with fb = sbc*t + ibv
    # where t < d; then residual vs y1, square + accumulate on scalar engine.
    fa = work.tile([P, F, Y], F32, tag=CFG["fa_tag"])
    aeng = nc.gpsimd if CFG["fa_g"] else nc.vector
    aeng.tensor_tensor(out=fa[:], in0=b3(sac), in1=tb, op=OP.mult)
    aeng.tensor_tensor(out=fa[:], in0=fa[:], in1=b3(ia2f), op=OP.add)
    fb = work.tile([P, F, Y], F32, tag="M")
    feng = nc.gpsimd if CFG["fb_g"] else nc.vector
    feng.tensor_tensor(out=fb[:], in0=b3(sbc), in1=tb, op=OP.mult)
    feng.tensor_tensor(out=fb[:], in0=fb[:], in1=b3(ibv), op=OP.add)
    nc.vector.copy_predicated(out=fa[:], mask=maskB[:].bitcast(mybir.dt.int32),
                              data=fb[:])
    reng = nc.gpsimd if CFG["r_g"] else nc.vector
    reng.tensor_tensor(out=fa[:], in0=fa[:], in1=y1pt, op=OP.subtract)
    nc.scalar.activation(out=fa[:], in_=fa[:], func=AF.Square,
                         accum_out=partial[:, c:c + 1])


def build_core_program():
    """Build the per-core Bass program (same program on all 8 cores)."""
    from contextlib import ExitStack

    nc = bacc.Bacc(trn_type="TRN2")
    y0d = nc.dram_tensor("y0", [Y, NPIX], F32, kind="ExternalInput")
    y1d = nc.dram_tensor("y1", [Y, NPIX], F32, kind="ExternalInput")
    outd = nc.dram_tensor("partial", [P, NCHUNK], F32, kind="ExternalOutput")

    with tile.TileContext(nc) as tc, ExitStack() as ctx:
        singles = ctx.enter_context(tc.tile_pool(name="singles", bufs=1))
        io = ctx.enter_context(tc.tile_pool(name="io", bufs=CFG["io_bufs"]))
        work = ctx.enter_context(tc.tile_pool(name="work",
                                              bufs=CFG["work_bufs"]))
        sm = ctx.enter_context(tc.tile_pool(name="sm", bufs=CFG["sm_bufs"]))

        # constants: z (bf16, 1 except 0 at t=0 of each pixel), t tile (bf16;
        # values 0..29 are exact, engines upconvert to fp32 internally)
        z = singles.tile([P, F, Y], BF16)
        nc.vector.memset(z[:], 1.0)
        nc.vector.memset(z[:, :, 0:1], 0.0)
        trow_i = sm.tile([P, Y], mybir.dt.int32, tag="trow_i")
        nc.gpsimd.iota(trow_i[:], pattern=[[1, Y]], base=0, channel_multiplier=0)
        trow = sm.tile([P, Y], F32, tag="trow")
        nc.vector.tensor_copy(trow[:], trow_i[:])
        tvec = singles.tile([P, F, Y], BF16)
        nc.vector.tensor_copy(
            tvec[:], trow[:][:, None, :].broadcast_to([P, F, Y]))
        tb = tvec[:]
        partial = singles.tile([P, NCHUNK], F32)

        pools = (io, work, sm)
        for c in range(NCHUNK):
            _emit_chunk(nc, pools, c, y0d, y1d, z, tb, partial)

        nc.sync.dma_start(out=outd[:, :], in_=partial[:])

    nc.finalize()   # Bacc: runs reg-alloc + the 1-wait sync-split lowering
    return nc


_NC = None


def _get_nc():
    global _NC
    if _NC is None:
        _NC = build_core_program()
    return _NC


def _make_in_maps(out):
    out = np.ascontiguousarray(out, dtype=np.float32)
    assert out.shape == (B, 2, Y, H, W), out.shape
    return [
        {
            "y0": out[b, 0].reshape(Y, NPIX),
            "y1": out[b, 1].reshape(Y, NPIX),
        }
        for b in range(B)
    ]


def kernel(out, target=None, **_ignored):
    """Full-input entry point: shards batch over 8 cores, returns scalar loss."""
    nc = _get_nc()
    in_maps = _make_in_maps(out)
    res = run_bass_kernel_spmd(nc, in_maps, core_ids=list(range(B)))
    total = sum(r["partial"].astype(np.float64).sum() for r in res.results)
    loss = total / float(B * Y * NPIX)
    return np.float32(loss)



# revision 6
# speedup vs baseline: 2.1056x; 2.1056x over previous
"""Trainium2 Bass kernel for DisturbanceRegressionLoss2Heads (v2).

Reference computation (per batch element b, per pixel):
  y0 = out[b,0] (Y=30 steps), y1 = out[b,1]
  diff = [-7, 0, y0[2]-y0[1], ..., y0[28]-y0[27], 0]
  d = argmin(diff) (first min)
  piecewise OLS fit of y0 on t<d (x=t) and t>=d (x=t-d); fitted slopes
  clipped to [0,2], intercepts clipped to [0,100]
  loss = mean((fitted - y1)^2)

v2 strategy (vs the strided-view v1): pure data parallel over batch (8
cores).  Per core the pixel axis lives on partitions/free-inner and the
30-step time axis is a FREE-OUTER axis, so every big tensor op is
contiguous (innermost step 1) and runs in the DVE's bf16 2x perf mode.
 - argmin: diff + two in-place min-trees with a +BIG pad; first-min index
   falls out of min(32*[D!=m] + t) with no extra select.
 - per-pixel sums (sy, sty, syb, styb) of centered y0: three masked/
   t-weighted products written into a packed [P,32,4F] tile, one in-place
   add-tree (last two levels fp32).
 - OLS epilogue on [P,F] smalls in fp32 (centering y0 by -40 kills the
   bf16 cancellation error in cov; intercepts add the 40 back).
 - residual r = (fa-y1) + maskB*(fb-fa) materialized in bf16 (6 TT passes)
   and squared+accumulated by the scalar engine (accum_out).
Casts run on the scalar engine; memsets/iota on gpsimd; everything is
double-buffered across 4 chunks so DMA/scalar/gpsimd overlap the DVE.
Built with bacc.Bacc (its sync-split lowering is required on TRN2).
"""

import numpy as np

import concourse.bacc as bacc
import concourse.tile as tile
from concourse import mybir
from concourse.bass_utils import run_bass_kernel_spmd

F32 = mybir.dt.float32
BF16 = mybir.dt.bfloat16
AX = mybir.AxisListType
OP = mybir.AluOpType
AF = mybir.ActivationFunctionType

B = 8
Y = 30
H = 256
W = 256
NPIX = H * W          # 65536 pixels per core
P = 128               # SBUF partitions
F = 128               # pixels per partition per chunk
CHUNK = P * F         # 16384 pixels
NCHUNK = NPIX // CHUNK  # 4
DIST = 7.0
MAXI = 100.0
BIG = 3.0e5           # +inf stand-in for min-tree pads (exact in bf16)
CEN = 40.0            # y0 centering constant

CFG = {
    "dd_g": False,       # diff TT on gpsimd instead of vector
    "y1_cast_dma": True,  # load y1 via SWDGE fp32->bf16 cast DMA
    "eplg_g": False,     # d-polynomial epilogue smalls on gpsimd
    "fast_recip": True,
}


def _emit_chunk(nc, pools, c, y0d, y1d, tb, cen_ap, partial):
    io, wk, sm = pools
    cs = c * CHUNK

    # ---- loads
    y0f = io.tile([P, Y, F], F32, tag="y0f")
    nc.sync.dma_start(
        out=y0f[:], in_=y0d[:, cs:cs + CHUNK].rearrange("y (p f) -> p y f", p=P))
    y1b = io.tile([P, Y, F], BF16, tag="y1b")
    if CFG["y1_cast_dma"]:
        nc.gpsimd.dma_start(
            out=y1b[:],
            in_=y1d[:, cs:cs + CHUNK].rearrange("y (p f) -> p y f", p=P))
    else:
        y1f = io.tile([P, Y, F], F32, tag="y1f")
        nc.sync.dma_start(
            out=y1f[:],
            in_=y1d[:, cs:cs + CHUNK].rearrange("y (p f) -> p y f", p=P))
        nc.scalar.activation(out=y1b[:], in_=y1f[:], func=AF.Copy)

    # ---- packed sums tile: slots along 4F = (y0b, w0, u0, mw0)
    PR = wk.tile([P, Y + 2, 4 * F], BF16, tag="PR", bufs=1)

    def prs(j, r0=0, r1=Y):  # slot view [P, r1-r0, F], row stride 4F
        return PR[:, r0:r1, j * F:(j + 1) * F]

    y0b = prs(0)
    # centered bf16 cast on the scalar engine
    nc.scalar.activation(out=y0b, in_=y0f[:], func=AF.Copy, bias=-CEN)
    nc.gpsimd.memset(PR[:, Y:Y + 2, :], 0.0)          # add-tree pad rows

    # ---- modified diff array D (rows: -7, 0, dd[2..28], 0, BIG, BIG)
    D = wk.tile([P, Y + 2, F], BF16, tag="D")
    nc.gpsimd.memset(D[:, 0:1, :], -DIST)
    nc.gpsimd.memset(D[:, 1:2, :], 0.0)
    nc.gpsimd.memset(D[:, Y - 1:Y, :], 0.0)
    nc.gpsimd.memset(D[:, Y:Y + 2, :], BIG)
    deng = nc.gpsimd if CFG["dd_g"] else nc.vector
    deng.tensor_tensor(out=D[:, 2:Y - 1, :], in0=prs(0, 2, Y - 1),
                       in1=prs(0, 1, Y - 2), op=OP.subtract)

    # ---- m = min over the 32 rows (first level out-of-place into TR)
    TR = wk.tile([P, 16, F], BF16, tag="TR")
    nc.vector.tensor_tensor(out=TR[:], in0=D[:, 0:16, :], in1=D[:, 16:32, :],
                            op=OP.min)
    for h in (8, 4, 2, 1):
        nc.vector.tensor_tensor(out=TR[:, 0:h, :], in0=TR[:, 0:h, :],
                                in1=TR[:, h:2 * h, :], op=OP.min)
    m_br = TR[:, 0:1, :].broadcast_to([P, Y, F])

    # ---- d = min over t of (32*[D != m] + t); the -7 sentinel row makes
    # d = first argmin index directly (valid values <= 29 < 32).
    nc.vector.tensor_tensor(out=D[:, 0:Y, :], in0=D[:, 0:Y, :], in1=m_br,
                            op=OP.not_equal)
    nc.vector.scalar_tensor_tensor(out=D[:, 0:Y, :], in0=D[:, 0:Y, :],
                                   scalar=32.0, in1=tb, op0=OP.mult,
                                   op1=OP.add)
    nc.vector.tensor_tensor(out=TR[:], in0=D[:, 0:16, :], in1=D[:, 16:32, :],
                            op=OP.min)
    for h in (8, 4, 2, 1):
        nc.vector.tensor_tensor(out=TR[:, 0:h, :], in0=TR[:, 0:h, :],
                                in1=TR[:, h:2 * h, :], op=OP.min)
    d_br = TR[:, 0:1, :].broadcast_to([P, Y, F])

    # ---- maskB = [t < d] (exact small-int compare in bf16), products
    mk = wk.tile([P, Y, F], BF16, tag="mk")
    nc.vector.tensor_tensor(out=mk[:], in0=tb, in1=d_br, op=OP.is_lt)
    nc.vector.tensor_tensor(out=prs(1), in0=tb, in1=prs(0), op=OP.mult)   # w0
    nc.vector.tensor_tensor(out=prs(2), in0=mk[:], in1=prs(0), op=OP.mult)  # u0
    nc.vector.tensor_tensor(out=prs(3), in0=mk[:], in1=prs(1), op=OP.mult)  # mw0

    # ---- add-tree over rows: bf16 in-place to 4 rows, then fp32 out
    nc.vector.tensor_tensor(out=PR[:, 0:16, :], in0=PR[:, 0:16, :],
                            in1=PR[:, 16:32, :], op=OP.add)
    for h in (8, 4):
        nc.vector.tensor_tensor(out=PR[:, 0:h, :], in0=PR[:, 0:h, :],
                                in1=PR[:, h:2 * h, :], op=OP.add)
    S4 = sm.tile([P, 2, 4 * F], F32, tag="S4")
    nc.vector.tensor_tensor(out=S4[:], in0=PR[:, 0:2, :], in1=PR[:, 2:4, :],
                            op=OP.add)
    SO = sm.tile([P, 4 * F], F32, tag="SO")
    nc.vector.tensor_tensor(out=SO[:], in0=S4[:, 0:1, :], in1=S4[:, 1:2, :],
                            op=OP.add)
    sy = SO[:, 0 * F:1 * F]
    sty = SO[:, 1 * F:2 * F]
    syb = SO[:, 2 * F:3 * F]
    styb = SO[:, 3 * F:4 * F]

    # ---- per-pixel OLS epilogue (fp32 smalls)
    def tt(name, a, bb, op):
        t = sm.tile([P, F], F32, tag=name)
        nc.vector.tensor_tensor(out=t[:], in0=a, in1=bb, op=op)
        return t[:]

    df_t = sm.tile([P, F], F32, tag="df")
    nc.vector.tensor_copy(df_t[:], TR[:, 0:1, :].rearrange("p o f -> p (o f)"))
    df = df_t[:]
    na_t = sm.tile([P, F], F32, tag="na")
    nc.scalar.activation(out=na_t[:], in_=df, func=AF.Copy, bias=float(Y),
                         scale=-1.0)
    na = na_t[:]

    eng = nc.gpsimd if CFG["eplg_g"] else nc.vector
    RIN = sm.tile([P, 4 * F], F32, tag="RIN")
    # slots: nbs=max(d,1), na, mvb=max(varb,1), mva=max(vara,1)
    nc.vector.tensor_scalar(out=RIN[:, 0:F], in0=df, scalar1=1.0, scalar2=None,
                            op0=OP.max)
    nc.vector.tensor_copy(RIN[:, F:2 * F], na)
    d2 = tt("d2", df, df, OP.mult)
    eng.scalar_tensor_tensor(out=d2, in0=d2, scalar=1.0, in1=df,
                             op0=OP.subtract, op1=OP.mult)      # d^3-d
    nc.vector.tensor_scalar(out=RIN[:, 2 * F:3 * F], in0=d2,
                            scalar1=1.0 / 12.0, scalar2=1.0,
                            op0=OP.mult, op1=OP.max)
    na2 = tt("na2", na, na, OP.mult)
    eng.scalar_tensor_tensor(out=na2, in0=na2, scalar=1.0, in1=na,
                             op0=OP.subtract, op1=OP.mult)
    nc.vector.tensor_scalar(out=RIN[:, 3 * F:4 * F], in0=na2,
                            scalar1=1.0 / 12.0, scalar2=1.0,
                            op0=OP.mult, op1=OP.max)
    RO = sm.tile([P, 4 * F], F32, tag="RO")
    if CFG["fast_recip"]:
        nc.vector.reciprocal_approx_fast(out=RO[:], in_=RIN[:])
    else:
        nc.vector.reciprocal(out=RO[:], in_=RIN[:])
    rnb = RO[:, 0:F]
    rna = RO[:, F:2 * F]
    rvb = RO[:, 2 * F:3 * F]
    rva = RO[:, 3 * F:4 * F]

    syA = tt("syA", sy, syb, OP.subtract)
    styA = tt("styA", sty, styb, OP.subtract)
    t5 = tt("t5", df, syA, OP.mult)
    nc.vector.tensor_tensor(out=t5, in0=styA, in1=t5, op=OP.subtract)
    sxya = t5                                    # sum (t-d)*y0c, t>=d

    myb = tt("myb", syb, rnb, OP.mult)           # centered means
    mya = tt("mya", syA, rna, OP.mult)
    mxb_t = sm.tile([P, F], F32, tag="mxb")
    nc.scalar.activation(out=mxb_t[:], in_=df, func=AF.Copy, bias=-0.5,
                         scale=0.5)
    mxa_t = sm.tile([P, F], F32, tag="mxa")
    nc.scalar.activation(out=mxa_t[:], in_=na, func=AF.Copy, bias=-0.5,
                         scale=0.5)
    mxb, mxa = mxb_t[:], mxa_t[:]

    covb = tt("covb", mxb, syb, OP.mult)
    nc.vector.tensor_tensor(out=covb, in0=styb, in1=covb, op=OP.subtract)
    cova = tt("cova", mxa, syA, OP.mult)
    nc.vector.tensor_tensor(out=cova, in0=sxya, in1=cova, op=OP.subtract)

    slb = tt("slb", covb, rvb, OP.mult)
    gate = sm.tile([P, F], F32, tag="gate")
    nc.vector.tensor_scalar(out=gate[:], in0=df, scalar1=2.0, scalar2=None,
                            op0=OP.is_ge)
    nc.vector.tensor_tensor(out=slb, in0=slb, in1=gate[:], op=OP.mult)
    sla = tt("sla", cova, rva, OP.mult)

    # intercepts (add the centering back, then clip to [0,100])
    ibv = tt("ibv", slb, mxb, OP.mult)
    nc.vector.tensor_tensor(out=ibv, in0=myb, in1=ibv, op=OP.subtract)
    nc.scalar.activation(out=ibv, in_=ibv, func=AF.Relu, bias=cen_ap)
    nc.vector.tensor_scalar(out=ibv, in0=ibv, scalar1=MAXI, scalar2=None,
                            op0=OP.min)
    iav = tt("iav", sla, mxa, OP.mult)
    nc.vector.tensor_tensor(out=iav, in0=mya, in1=iav, op=OP.subtract)
    nc.scalar.activation(out=iav, in_=iav, func=AF.Relu, bias=cen_ap)
    nc.vector.tensor_scalar(out=iav, in0=iav, scalar1=MAXI, scalar2=None,
                            op0=OP.min)

    sbc = sm.tile([P, F], F32, tag="sbc")
    nc.vector.tensor_scalar(out=sbc[:], in0=slb, scalar1=0.0, scalar2=2.0,
                            op0=OP.max, op1=OP.min)
    sac = sm.tile([P, F], F32, tag="sac")
    nc.vector.tensor_scalar(out=sac[:], in0=sla, scalar1=0.0, scalar2=2.0,
                            op0=OP.max, op1=OP.min)
    ca = tt("ca", sac[:], df, OP.mult)
    nc.vector.tensor_tensor(out=ca, in0=iav, in1=ca, op=OP.subtract)
    dls = tt("dls", sbc[:], sac[:], OP.subtract)   # slope delta
    dli = tt("dli", ibv, ca, OP.subtract)          # intercept delta

    # bf16 casts of the 4 line coefficients (scalar engine)
    cb4 = sm.tile([P, 4 * F], BF16, tag="cb4")
    nc.scalar.activation(out=cb4[:, 0:F], in_=sac[:], func=AF.Copy)
    nc.scalar.activation(out=cb4[:, F:2 * F], in_=ca, func=AF.Copy)
    nc.scalar.activation(out=cb4[:, 2 * F:3 * F], in_=dls, func=AF.Copy)
    nc.scalar.activation(out=cb4[:, 3 * F:4 * F], in_=dli, func=AF.Copy)

    def cbr(j):
        return cb4[:, j * F:(j + 1) * F][:, None, :].broadcast_to([P, Y, F])

    # ---- residual r = (sac*t + ca - y1) + maskB*(dls*t + dli); square+accum
    TA = wk.tile([P, Y, F], BF16, tag="D")
    TC = wk.tile([P, Y, F], BF16, tag="TR")
    nc.vector.tensor_tensor(out=TC[:], in0=cbr(1), in1=y1b[:], op=OP.subtract)
    nc.vector.tensor_tensor(out=TA[:], in0=cbr(0), in1=tb, op=OP.mult)
    nc.vector.tensor_tensor(out=TA[:], in0=TA[:], in1=TC[:], op=OP.add)
    nc.vector.tensor_tensor(out=TC[:], in0=cbr(2), in1=tb, op=OP.mult)
    nc.vector.tensor_tensor(out=TC[:], in0=TC[:], in1=cbr(3), op=OP.add)
    nc.vector.tensor_tensor(out=TC[:], in0=mk[:], in1=TC[:], op=OP.mult)
    nc.vector.tensor_tensor(out=TA[:], in0=TA[:], in1=TC[:], op=OP.add)
    nc.scalar.activation(out=TA[:], in_=TA[:], func=AF.Square,
                         accum_out=partial[:, c:c + 1])


def build_core_program():
    from contextlib import ExitStack

    nc = bacc.Bacc(trn_type="TRN2")
    y0d = nc.dram_tensor("y0", [Y, NPIX], F32, kind="ExternalInput")
    y1d = nc.dram_tensor("y1", [Y, NPIX], F32, kind="ExternalInput")
    outd = nc.dram_tensor("partial", [P, NCHUNK], F32, kind="ExternalOutput")

    with tile.TileContext(nc) as tc, ExitStack() as ctx:
        singles = ctx.enter_context(tc.tile_pool(name="singles", bufs=1))
        io = ctx.enter_context(tc.tile_pool(name="io", bufs=2))
        wk = ctx.enter_context(tc.tile_pool(name="wk", bufs=2))
        sm = ctx.enter_context(tc.tile_pool(name="sm", bufs=1))

        trow_i = singles.tile([P, Y], mybir.dt.int32)
        nc.gpsimd.iota(trow_i[:], pattern=[[1, Y]], base=0,
                       channel_multiplier=0)
        trow = singles.tile([P, Y], F32)
        nc.vector.tensor_copy(trow[:], trow_i[:])
        tvec = singles.tile([P, Y, F], BF16)
        nc.vector.tensor_copy(
            tvec[:], trow[:][:, :, None].broadcast_to([P, Y, F]))
        tb = tvec[:]
        partial = singles.tile([P, NCHUNK], F32)
        cen_t = singles.tile([P, 1], F32)
        nc.gpsimd.memset(cen_t[:], CEN)

        pools = (io, wk, sm)
        for c in range(NCHUNK):
            _emit_chunk(nc, pools, c, y0d, y1d, tb, cen_t[:], partial)

        nc.sync.dma_start(out=outd[:, :], in_=partial[:])

    nc.finalize()
    return nc


_NC = None


def _get_nc():
    global _NC
    if _NC is None:
        _NC = build_core_program()
    return _NC


def _make_in_maps(out):
    out = np.ascontiguousarray(out, dtype=np.float32)
    assert out.shape == (B, 2, Y, H, W), out.shape
    return [
        {
            "y0": out[b, 0].reshape(Y, NPIX),
            "y1": out[b, 1].reshape(Y, NPIX),
        }
        for b in range(B)
    ]


def kernel(out, target=None, **_ignored):
    """Full-input entry point: shards batch over 8 cores, returns scalar loss."""
    nc = _get_nc()
    in_maps = _make_in_maps(out)
    res = run_bass_kernel_spmd(nc, in_maps, core_ids=list(range(B)))
    total = sum(r["partial"].astype(np.float64).sum() for r in res.results)
    loss = total / float(B * Y * NPIX)
    return np.float32(loss)
